# revision 12
# baseline (speedup 1.0000x reference)
"""Bass/Trainium2 kernel for nn_F_Loss_65446711656630.

Strategy (data-parallel over N, 8 cores):
  - Host: per core, sort the core's 8192 rows by class id and transpose to
    [512 features, 8192 rows] (contiguous).  After sorting, almost every
    128-row chunk is single-class.
  - Device (static kernel, no data-dependent structure): stream [128, 1024]
    pieces; per piece compute per-128-row-chunk partial sums (DVE
    multi-chunk reduce via 3D access pattern) and partial sums-of-squares
    (ACT square + DVE reduce).  Pure f32, no matmul, PE idle; this is the
    memory-bound part (128 MiB streamed at HBM rate).
  - Host: per-class stats = sum of single-class chunk partials (fp64)
    + direct numpy sums for the few class-boundary chunks; then the tiny
    O(C^2 D) pairwise betainc/top-k stage (C=16) on CPU.
"""

import numpy as np

C = 16
D = 512
N = 65536
NCORES = 8
ROWS = N // NCORES          # 8192 rows per core
P = 128                     # SBUF partitions
PIECE = 1024                # rows per DMA piece
X = 128                     # rows per reduction chunk
NBLK = D // P               # 4 feature blocks
NPIECE = ROWS // PIECE      # 8 pieces per block
NCHUNK = ROWS // X          # 64 chunks per core
CPP = PIECE // X            # 8 chunks per piece
XMIN, XMAX = 1e-37, 1.0 - 1e-5

_NC_CACHE = {}


def _build_nc():
    """Per-core SPMD program: chunkwise partial sums / sums-of-squares.

    Inputs:  "ht"   [512, 8192]  f32  (sorted, transposed hidden)
    Outputs: "hsum" [128, 256]   f32  (hsum[f, b*64+g] = sum over chunk g's
                                       rows of feature b*128+f)
             "ssum" [128, 256]   f32  (same for squares)
    """
    import concourse.tile as tile
    from concourse import bacc, mybir

    f32 = mybir.dt.float32

    nc = bacc.Bacc("TRN2", target_bir_lowering=False, debug=False,
                   num_devices=NCORES)
    # Host pre-tiles to [NBLK*NPIECE, 128, PIECE] so every piece is one
    # fully-contiguous 512 KiB DMA (strided pieces measured ~30% slower).
    ht = nc.declare_dram_parameter("ht", [NBLK * NPIECE, P, PIECE], f32,
                                   isOutput=False)
    hsum = nc.declare_dram_parameter("hsum", [P, NBLK * NCHUNK], f32, isOutput=True)
    ssum = nc.declare_dram_parameter("ssum", [P, NBLK * NCHUNK], f32, isOutput=True)

    with tile.TileContext(nc) as tc:
        with (
            tc.tile_pool(name="pc", bufs=6) as piece_pool,
            tc.tile_pool(name="sq", bufs=6) as sq_pool,
            tc.tile_pool(name="acc", bufs=1) as acc_pool,
        ):
            hpart = acc_pool.tile([P, NBLK * NCHUNK], f32, tag="hpart")
            spart = acc_pool.tile([P, NBLK * NCHUNK], f32, tag="spart")

            # One full pass over the data costs ~38 us on DVE or ACT, and
            # three passes are needed (square, reduce-h, reduce-sq).  ACT
            # takes the square (emitting bf16 — squares tolerate 8 mantissa
            # bits; sumsq rel err ~1e-5), DVE takes reduce-h in f32 plus
            # reduce-sq at the 2x 16-bit rate.
            bf16 = mybir.dt.bfloat16
            for i in range(NBLK * NPIECE):
                t = piece_pool.tile([P, PIECE], f32)
                nc.sync.dma_start(t[:], ht[i])
                sq = sq_pool.tile([P, PIECE], bf16)
                nc.scalar.square(sq[:], t[:])

                base = i * CPP
                t3 = t[:].rearrange("p (c x) -> p c x", x=X)
                s3 = sq[:].rearrange("p (c x) -> p c x", x=X)
                nc.vector.reduce_sum(
                    hpart[:, base:base + CPP], t3, axis=mybir.AxisListType.X)
                nc.vector.reduce_sum(
                    spart[:, base:base + CPP], s3, axis=mybir.AxisListType.X)

            nc.sync.dma_start(hsum[:], hpart[:])
            nc.sync.dma_start(ssum[:], spart[:])
    nc.compile()
    return nc


def _get_nc():
    if "nc" not in _NC_CACHE:
        _NC_CACHE["nc"] = _build_nc()
    return _NC_CACHE["nc"]


def _prep_core(hidden_k, ids_k):
    """Sort rows by class, transpose; classify chunks; boundary-row stats."""
    order = np.argsort(ids_k, kind="stable")
    ids_sorted = ids_k[order]
    hs = hidden_k[order]                         # [8192, 512] f32, sorted
    # piece (b,p) holds features b*128..+128 x rows p*1024..+1024, contiguous
    T = np.ascontiguousarray(
        hs.reshape(NPIECE, PIECE, NBLK, P).transpose(2, 0, 3, 1)
    ).reshape(NBLK * NPIECE, P, PIECE)           # [32, 128, 1024] f32

    cm = ids_sorted.reshape(NCHUNK, X)
    interior = cm[:, 0] == cm[:, -1]             # single-class chunk?
    chunk_class = np.where(interior, cm[:, 0], -1).astype(np.int64)

    bsum = np.zeros((C, D), dtype=np.float64)
    bsq = np.zeros((C, D), dtype=np.float64)
    if not interior.all():
        bmask = np.repeat(~interior, X)
        brows = hs[bmask].astype(np.float64)     # rows in boundary chunks
        bids = ids_sorted[bmask]
        for q in np.unique(bids):
            sel = brows[bids == q]
            bsum[q] = sel.sum(axis=0)
            bsq[q] = (sel * sel).sum(axis=0)
    return T, chunk_class, bsum, bsq


def _device_stats(hidden, ids, **run_kwargs):
    """Returns (sums[C,D], sumsq[C,D]) float64, plus the raw run result."""
    from concourse import bass_utils

    nc = _get_nc()
    in_maps = []
    chunk_classes = []
    sums = np.zeros((C, D), dtype=np.float64)
    sumsq = np.zeros((C, D), dtype=np.float64)
    for k in range(NCORES):
        rows = slice(k * ROWS, (k + 1) * ROWS)
        T, cls, bsum, bsq = _prep_core(hidden[rows], ids[rows])
        in_maps.append({"ht": T})
        chunk_classes.append(cls)
        sums += bsum
        sumsq += bsq

    res = bass_utils.run_bass_kernel_spmd(nc, in_maps, list(range(NCORES)), **run_kwargs)

    for k in range(NCORES):
        # dev output [128, 4*64]: col b*64+g, row f -> feature b*128+f, chunk g
        hp = res.results[k]["hsum"].astype(np.float64)
        sp = res.results[k]["ssum"].astype(np.float64)
        # -> [chunk, feature]
        hp = hp.reshape(P, NBLK, NCHUNK).transpose(2, 1, 0).reshape(NCHUNK, D)
        sp = sp.reshape(P, NBLK, NCHUNK).transpose(2, 1, 0).reshape(NCHUNK, D)
        cls = chunk_classes[k]
        sel = cls >= 0
        oh = (cls[sel, None] == np.arange(C)[None, :]).astype(np.float64)  # [g, C]
        sums += oh.T @ hp[sel]
        sumsq += oh.T @ sp[sel]
    return sums, sumsq, res


def _pairwise_loss(counts, sums, sumsq, d):
    """The tiny O(C^2 D) stage on host CPU.

    Runs in float32 with the same jax ops as the reference: at these extreme
    betainc parameters (b ~ 8190, x ~ 1e-5) jax's f32 betainc differs from
    the true (f64) value by ~1e-3, so matching the reference requires
    replicating its f32 numerics, not improving on them.
    """
    import jax
    import jax.numpy as jnp

    cpu = jax.devices("cpu")[0]
    with jax.default_device(cpu):
        counts64 = counts.astype(np.float64)
        means64 = sums / counts64[:, None]
        withins64 = sumsq - counts64[:, None] * means64**2
        counts = jnp.asarray(counts64, jnp.float32)               # [C]
        means = jnp.asarray(means64, jnp.float32)                 # [C, D]
        withins = jnp.asarray(withins64, jnp.float32)             # [C, D]
        half_diff = (means[:, None, :] - means[None, :, :]) * 0.5
        pair_counts = counts[:, None] + counts[None, :]
        pair_between = half_diff * half_diff * pair_counts[:, :, None]
        pair_within = withins[:, None, :] + withins[None, :, :]
        d2 = pair_counts - 2.0
        d2 = jnp.where(d2 == 0.0, 1e-5, d2)
        x = pair_between / (pair_between + pair_within)
        x = jnp.clip(x, XMIN, XMAX)
        a = jnp.full_like(x, 0.5)
        b = jnp.broadcast_to((d2 * 0.5)[:, :, None], x.shape)
        xbetainc = jax.scipy.special.betainc(a, b, x)             # [C, C, D]
        top_k, _ = jax.lax.top_k(xbetainc, int(d))                # [C, C, d]
        per_pair = jnp.sum(jnp.log(top_k), axis=-1)               # [C, C]
        mask = jnp.triu(jnp.ones((C, C), dtype=bool), k=1)
        total = jnp.sum(jnp.where(mask, per_pair, jnp.zeros_like(per_pair)))
        return float(-total)


def kernel(hidden, batch_ids, d):
    hidden = np.asarray(hidden, dtype=np.float32)
    ids = np.asarray(batch_ids).astype(np.int64)
    assert hidden.shape == (N, D), hidden.shape

    counts = np.bincount(ids, minlength=C).astype(np.float64)
    sums, sumsq, _ = _device_stats(hidden, ids)
    total = _pairwise_loss(counts, sums, sumsq, int(np.asarray(d)))
    return np.array(total, dtype=np.float32)


# revision 13
# speedup vs baseline: 1.5336x; 1.5336x over previous
"""Bass/Trainium2 kernel for nn_F_Loss_65446711656630.

Strategy (data-parallel over N, 8 cores):
  - Host: GLOBAL stable sort of all rows by class id, then slice 8192 rows
    per core and transpose to [512 features x 8192 rows] contiguous pieces.
    After a global sort each core spans only ~2 classes, so class
    transitions are rare at any granularity.
  - Device (static kernel): stream 32 pieces of [128, 1024]; per piece
      * DVE:  per-128-row-chunk partial sums of h (one multi-chunk
              TensorReduce per piece, 3D access pattern)
      * ACT:  square with accum_out -> per-piece partial sum of h^2
              (the square pass itself yields the sumsq reduction, so no
              second DVE reduce pass is needed)
    One full data pass costs ~35-39 us on either engine; DMA (~48 us for
    16 MiB/core) is the roofline.
  - Host: per-class stats from single-class chunk/piece partials (fp64)
    + direct numpy sums for the few transition chunks/pieces; then the
    tiny O(C^2 D) pairwise betainc/top-k stage in f32 jax on CPU
    (mirroring the reference's numerics exactly).
"""

import numpy as np

C = 16
D = 512
N = 65536
NCORES = 8
ROWS = N // NCORES          # 8192 rows per core
P = 128                     # SBUF partitions
PIECE = 1024                # rows per DMA piece / sumsq granule
X = 128                     # rows per sums granule (DVE reduce chunk)
NBLK = D // P               # 4 feature blocks
NPIECE = ROWS // PIECE      # 8 pieces per block
NCHUNK = ROWS // X          # 64 chunks per core
CPP = PIECE // X            # 8 chunks per piece
XMIN, XMAX = 1e-37, 1.0 - 1e-5

_NC_CACHE = {}


def _build_nc():
    """Per-core SPMD program.

    Inputs:  "ht"   [32, 128, 1024] f32 (piece (b,p) at index b*8+p holds
                                         features b*128..+128 x rows
                                         p*1024..+1024, contiguous)
    Outputs: "hsum" [128, 256] f32  (hsum[f, b*64+g] = sum over 128-row
                                     chunk g of feature b*128+f)
             "ssum" [128, 32]  f32  (ssum[f, b*8+p]  = sum over piece p's
                                     1024 rows of feature (b*128+f)^2)
    """
    import concourse.tile as tile
    from concourse import bacc, mybir

    f32 = mybir.dt.float32

    nc = bacc.Bacc("TRN2", target_bir_lowering=False, debug=False,
                   num_devices=NCORES)
    ht = nc.declare_dram_parameter("ht", [NBLK * NPIECE, P, PIECE], f32,
                                   isOutput=False)
    hsum = nc.declare_dram_parameter("hsum", [P, NBLK * NCHUNK], f32, isOutput=True)
    ssum = nc.declare_dram_parameter("ssum", [P, NBLK * NPIECE], f32, isOutput=True)

    with tile.TileContext(nc) as tc:
        with (
            tc.tile_pool(name="pc", bufs=6) as piece_pool,
            tc.tile_pool(name="sq", bufs=3) as sq_pool,
            tc.tile_pool(name="acc", bufs=1) as acc_pool,
        ):
            hpart = acc_pool.tile([P, NBLK * NCHUNK], f32, tag="hpart")
            spart = acc_pool.tile([P, NBLK * NPIECE], f32, tag="spart")

            for i in range(NBLK * NPIECE):
                t = piece_pool.tile([P, PIECE], f32)
                nc.sync.dma_start(t[:], ht[i])

                # ACT: square (scratch) + free-dim accumulate -> piece sumsq
                sq = sq_pool.tile([P, PIECE], f32)
                nc.scalar.activation(
                    sq[:], t[:], mybir.ActivationFunctionType.Square,
                    accum_out=spart[:, i:i + 1])

                # DVE: one multi-chunk reduce -> 8 chunk sums of h
                base = i * CPP
                t3 = t[:].rearrange("p (c x) -> p c x", x=X)
                nc.vector.reduce_sum(
                    hpart[:, base:base + CPP], t3, axis=mybir.AxisListType.X)

            nc.sync.dma_start(hsum[:], hpart[:])
            nc.sync.dma_start(ssum[:], spart[:])
    nc.compile()
    return nc


def _get_nc():
    if "nc" not in _NC_CACHE:
        _NC_CACHE["nc"] = _build_nc()
    return _NC_CACHE["nc"]


def _granule_classes(ids_sorted, size):
    """Per-granule class id, or -1 if the granule spans a class boundary."""
    g = ids_sorted.reshape(-1, size)
    pure = g[:, 0] == g[:, -1]
    return np.where(pure, g[:, 0], -1).astype(np.int64)


def _prep_core(hs_k, ids_k):
    """hs_k/ids_k already globally sorted. Returns device input + host fixups."""
    T = np.ascontiguousarray(
        hs_k.reshape(NPIECE, PIECE, NBLK, P).transpose(2, 0, 3, 1)
    ).reshape(NBLK * NPIECE, P, PIECE)           # [32, 128, 1024] f32

    chunk_cls = _granule_classes(ids_k, X)       # [64]
    piece_cls = _granule_classes(ids_k, PIECE)   # [8]

    bsum = np.zeros((C, D), dtype=np.float64)
    bsq = np.zeros((C, D), dtype=np.float64)
    # transition chunks: host computes their per-class h sums
    if (chunk_cls < 0).any():
        m = np.repeat(chunk_cls < 0, X)
        rows, rids = hs_k[m].astype(np.float64), ids_k[m]
        for q in np.unique(rids):
            bsum[q] += rows[rids == q].sum(axis=0)
    # transition pieces: host computes their per-class h^2 sums
    if (piece_cls < 0).any():
        m = np.repeat(piece_cls < 0, PIECE)
        rows, rids = hs_k[m].astype(np.float64), ids_k[m]
        for q in np.unique(rids):
            sel = rows[rids == q]
            bsq[q] += (sel * sel).sum(axis=0)
    return T, chunk_cls, piece_cls, bsum, bsq


def _device_stats(hidden, ids, **run_kwargs):
    """Returns (sums[C,D], sumsq[C,D]) float64, plus the raw run result."""
    from concourse import bass_utils

    nc = _get_nc()

    order = np.argsort(ids, kind="stable")       # GLOBAL sort by class
    ids_s = ids[order]
    hs = hidden[order]

    in_maps = []
    meta = []
    sums = np.zeros((C, D), dtype=np.float64)
    sumsq = np.zeros((C, D), dtype=np.float64)
    for k in range(NCORES):
        rows = slice(k * ROWS, (k + 1) * ROWS)
        T, ccls, pcls, bsum, bsq = _prep_core(hs[rows], ids_s[rows])
        in_maps.append({"ht": T})
        meta.append((ccls, pcls))
        sums += bsum
        sumsq += bsq

    res = bass_utils.run_bass_kernel_spmd(nc, in_maps, list(range(NCORES)), **run_kwargs)

    eye = np.arange(C)[None, :]
    for k in range(NCORES):
        ccls, pcls = meta[k]
        hp = res.results[k]["hsum"].astype(np.float64)
        sp = res.results[k]["ssum"].astype(np.float64)
        # [128, b, g] -> [g, b, 128] -> [granule, feature]
        hp = hp.reshape(P, NBLK, NCHUNK).transpose(2, 1, 0).reshape(NCHUNK, D)
        sp = sp.reshape(P, NBLK, NPIECE).transpose(2, 1, 0).reshape(NPIECE, D)
        cm = ccls >= 0
        sums += ((ccls[cm, None] == eye).astype(np.float64)).T @ hp[cm]
        pm = pcls >= 0
        sumsq += ((pcls[pm, None] == eye).astype(np.float64)).T @ sp[pm]
    return sums, sumsq, res


def _pairwise_loss(counts, sums, sumsq, d):
    """The tiny O(C^2 D) stage on host CPU.

    Runs in float32 with the same jax ops as the reference: at these extreme
    betainc parameters (b ~ 8190, x ~ 1e-5) jax's f32 betainc differs from
    the true (f64) value by ~1e-3, so matching the reference requires
    replicating its f32 numerics, not improving on them.
    """
    import jax
    import jax.numpy as jnp

    cpu = jax.devices("cpu")[0]
    with jax.default_device(cpu):
        counts64 = counts.astype(np.float64)
        means64 = sums / counts64[:, None]
        withins64 = sumsq - counts64[:, None] * means64**2
        counts = jnp.asarray(counts64, jnp.float32)               # [C]
        means = jnp.asarray(means64, jnp.float32)                 # [C, D]
        withins = jnp.asarray(withins64, jnp.float32)             # [C, D]
        half_diff = (means[:, None, :] - means[None, :, :]) * 0.5
        pair_counts = counts[:, None] + counts[None, :]
        pair_between = half_diff * half_diff * pair_counts[:, :, None]
        pair_within = withins[:, None, :] + withins[None, :, :]
        d2 = pair_counts - 2.0
        d2 = jnp.where(d2 == 0.0, 1e-5, d2)
        x = pair_between / (pair_between + pair_within)
        x = jnp.clip(x, XMIN, XMAX)
        a = jnp.full_like(x, 0.5)
        b = jnp.broadcast_to((d2 * 0.5)[:, :, None], x.shape)
        xbetainc = jax.scipy.special.betainc(a, b, x)             # [C, C, D]
        top_k, _ = jax.lax.top_k(xbetainc, int(d))                # [C, C, d]
        per_pair = jnp.sum(jnp.log(top_k), axis=-1)               # [C, C]
        mask = jnp.triu(jnp.ones((C, C), dtype=bool), k=1)
        total = jnp.sum(jnp.where(mask, per_pair, jnp.zeros_like(per_pair)))
        return float(-total)


def kernel(hidden, batch_ids, d):
    hidden = np.asarray(hidden, dtype=np.float32)
    ids = np.asarray(batch_ids).astype(np.int64)
    assert hidden.shape == (N, D), hidden.shape

    counts = np.bincount(ids, minlength=C).astype(np.float64)
    sums, sumsq, _ = _device_stats(hidden, ids)
    total = _pairwise_loss(counts, sums, sumsq, int(np.asarray(d)))
    return np.array(total, dtype=np.float32)


# revision 14
# speedup vs baseline: 1.5609x; 1.0178x over previous
"""Bass/Trainium2 kernel for nn_F_Loss_65446711656630.

Strategy (data-parallel over N, 8 cores):
  - Host: GLOBAL stable sort of all rows by class id, then slice 8192 rows
    per core and transpose to [512 features x 8192 rows] contiguous pieces.
    After a global sort each core spans only ~2 classes, so class
    transitions are rare at any granularity.
  - Device (static kernel): stream 32 pieces of [128, 1024]; per piece
      * DVE:  per-128-row-chunk partial sums of h (one multi-chunk
              TensorReduce per piece, 3D access pattern)
      * ACT:  square with accum_out -> per-piece partial sum of h^2
              (the square pass itself yields the sumsq reduction, so no
              second DVE reduce pass is needed)
    One full data pass costs ~35-39 us on either engine; DMA (~48 us for
    16 MiB/core) is the roofline.
  - Host: per-class stats from single-class chunk/piece partials (fp64)
    + direct numpy sums for the few transition chunks/pieces; then the
    tiny O(C^2 D) pairwise betainc/top-k stage in f32 jax on CPU
    (mirroring the reference's numerics exactly).
"""

import numpy as np

C = 16
D = 512
N = 65536
NCORES = 8
ROWS = N // NCORES          # 8192 rows per core
P = 128                     # SBUF partitions
PIECE = 2048                # rows per DMA piece / sumsq granule
X = 128                     # rows per sums granule (DVE reduce chunk)
NBLK = D // P               # 4 feature blocks
NPIECE = ROWS // PIECE      # 8 pieces per block
NCHUNK = ROWS // X          # 64 chunks per core
CPP = PIECE // X            # 8 chunks per piece
XMIN, XMAX = 1e-37, 1.0 - 1e-5

_NC_CACHE = {}


def _build_nc():
    """Per-core SPMD program.

    Inputs:  "ht"   [16, 128, 2048] f32 (piece (b,p) at index b*4+p holds
                                         features b*128..+128 x rows
                                         p*2048..+2048, contiguous)
    Outputs: "hsum" [128, 256] f32  (hsum[f, b*64+g] = sum over 128-row
                                     chunk g of feature b*128+f)
             "ssum" [128, 16]  f32  (ssum[f, b*4+p]  = sum over piece p's
                                     2048 rows of feature (b*128+f)^2)
    """
    import concourse.tile as tile
    from concourse import bacc, mybir

    f32 = mybir.dt.float32

    nc = bacc.Bacc("TRN2", target_bir_lowering=False, debug=False,
                   num_devices=NCORES)
    ht = nc.declare_dram_parameter("ht", [NBLK * NPIECE, P, PIECE], f32,
                                   isOutput=False)
    hsum = nc.declare_dram_parameter("hsum", [P, NBLK * NCHUNK], f32, isOutput=True)
    ssum = nc.declare_dram_parameter("ssum", [P, NBLK * NPIECE], f32, isOutput=True)

    with tile.TileContext(nc) as tc:
        with (
            tc.tile_pool(name="pc", bufs=8) as piece_pool,
            tc.tile_pool(name="sq", bufs=3) as sq_pool,
            tc.tile_pool(name="acc", bufs=1) as acc_pool,
        ):
            hpart = acc_pool.tile([P, NBLK * NCHUNK], f32, tag="hpart")
            spart = acc_pool.tile([P, NBLK * NPIECE], f32, tag="spart")

            for i in range(NBLK * NPIECE):
                t = piece_pool.tile([P, PIECE], f32)
                nc.sync.dma_start(t[:], ht[i])

                # ACT: square (scratch) + free-dim accumulate -> piece sumsq
                sq = sq_pool.tile([P, PIECE], f32)
                nc.scalar.activation(
                    sq[:], t[:], mybir.ActivationFunctionType.Square,
                    accum_out=spart[:, i:i + 1])

                # DVE: one multi-chunk reduce -> 8 chunk sums of h
                base = i * CPP
                t3 = t[:].rearrange("p (c x) -> p c x", x=X)
                nc.vector.reduce_sum(
                    hpart[:, base:base + CPP], t3, axis=mybir.AxisListType.X)

            nc.sync.dma_start(hsum[:], hpart[:])
            nc.sync.dma_start(ssum[:], spart[:])
    nc.compile()
    return nc


def _get_nc():
    if "nc" not in _NC_CACHE:
        _NC_CACHE["nc"] = _build_nc()
    return _NC_CACHE["nc"]


def _granule_classes(ids_sorted, size):
    """Per-granule class id, or -1 if the granule spans a class boundary."""
    g = ids_sorted.reshape(-1, size)
    pure = g[:, 0] == g[:, -1]
    return np.where(pure, g[:, 0], -1).astype(np.int64)


def _prep_core(hs_k, ids_k):
    """hs_k/ids_k already globally sorted. Returns device input + host fixups."""
    T = np.ascontiguousarray(
        hs_k.reshape(NPIECE, PIECE, NBLK, P).transpose(2, 0, 3, 1)
    ).reshape(NBLK * NPIECE, P, PIECE)           # [16, 128, 2048] f32

    chunk_cls = _granule_classes(ids_k, X)       # [64]
    piece_cls = _granule_classes(ids_k, PIECE)   # [8]

    bsum = np.zeros((C, D), dtype=np.float64)
    bsq = np.zeros((C, D), dtype=np.float64)
    # transition chunks: host computes their per-class h sums
    if (chunk_cls < 0).any():
        m = np.repeat(chunk_cls < 0, X)
        rows, rids = hs_k[m].astype(np.float64), ids_k[m]
        for q in np.unique(rids):
            bsum[q] += rows[rids == q].sum(axis=0)
    # transition pieces: host computes their per-class h^2 sums
    if (piece_cls < 0).any():
        m = np.repeat(piece_cls < 0, PIECE)
        rows, rids = hs_k[m].astype(np.float64), ids_k[m]
        for q in np.unique(rids):
            sel = rows[rids == q]
            bsq[q] += (sel * sel).sum(axis=0)
    return T, chunk_cls, piece_cls, bsum, bsq


def _device_stats(hidden, ids, **run_kwargs):
    """Returns (sums[C,D], sumsq[C,D]) float64, plus the raw run result."""
    from concourse import bass_utils

    nc = _get_nc()

    order = np.argsort(ids, kind="stable")       # GLOBAL sort by class
    ids_s = ids[order]
    hs = hidden[order]

    in_maps = []
    meta = []
    sums = np.zeros((C, D), dtype=np.float64)
    sumsq = np.zeros((C, D), dtype=np.float64)
    for k in range(NCORES):
        rows = slice(k * ROWS, (k + 1) * ROWS)
        T, ccls, pcls, bsum, bsq = _prep_core(hs[rows], ids_s[rows])
        in_maps.append({"ht": T})
        meta.append((ccls, pcls))
        sums += bsum
        sumsq += bsq

    res = bass_utils.run_bass_kernel_spmd(nc, in_maps, list(range(NCORES)), **run_kwargs)

    eye = np.arange(C)[None, :]
    for k in range(NCORES):
        ccls, pcls = meta[k]
        hp = res.results[k]["hsum"].astype(np.float64)
        sp = res.results[k]["ssum"].astype(np.float64)
        # [128, b, g] -> [g, b, 128] -> [granule, feature]
        hp = hp.reshape(P, NBLK, NCHUNK).transpose(2, 1, 0).reshape(NCHUNK, D)
        sp = sp.reshape(P, NBLK, NPIECE).transpose(2, 1, 0).reshape(NPIECE, D)
        cm = ccls >= 0
        sums += ((ccls[cm, None] == eye).astype(np.float64)).T @ hp[cm]
        pm = pcls >= 0
        sumsq += ((pcls[pm, None] == eye).astype(np.float64)).T @ sp[pm]
    return sums, sumsq, res


def _pairwise_loss(counts, sums, sumsq, d):
    """The tiny O(C^2 D) stage on host CPU.

    Runs in float32 with the same jax ops as the reference: at these extreme
    betainc parameters (b ~ 8190, x ~ 1e-5) jax's f32 betainc differs from
    the true (f64) value by ~1e-3, so matching the reference requires
    replicating its f32 numerics, not improving on them.
    """
    import jax
    import jax.numpy as jnp

    cpu = jax.devices("cpu")[0]
    with jax.default_device(cpu):
        counts64 = counts.astype(np.float64)
        means64 = sums / counts64[:, None]
        withins64 = sumsq - counts64[:, None] * means64**2
        counts = jnp.asarray(counts64, jnp.float32)               # [C]
        means = jnp.asarray(means64, jnp.float32)                 # [C, D]
        withins = jnp.asarray(withins64, jnp.float32)             # [C, D]
        half_diff = (means[:, None, :] - means[None, :, :]) * 0.5
        pair_counts = counts[:, None] + counts[None, :]
        pair_between = half_diff * half_diff * pair_counts[:, :, None]
        pair_within = withins[:, None, :] + withins[None, :, :]
        d2 = pair_counts - 2.0
        d2 = jnp.where(d2 == 0.0, 1e-5, d2)
        x = pair_between / (pair_between + pair_within)
        x = jnp.clip(x, XMIN, XMAX)
        a = jnp.full_like(x, 0.5)
        b = jnp.broadcast_to((d2 * 0.5)[:, :, None], x.shape)
        xbetainc = jax.scipy.special.betainc(a, b, x)             # [C, C, D]
        top_k, _ = jax.lax.top_k(xbetainc, int(d))                # [C, C, d]
        per_pair = jnp.sum(jnp.log(top_k), axis=-1)               # [C, C]
        mask = jnp.triu(jnp.ones((C, C), dtype=bool), k=1)
        total = jnp.sum(jnp.where(mask, per_pair, jnp.zeros_like(per_pair)))
        return float(-total)


def kernel(hidden, batch_ids, d):
    hidden = np.asarray(hidden, dtype=np.float32)
    ids = np.asarray(batch_ids).astype(np.int64)
    assert hidden.shape == (N, D), hidden.shape

    counts = np.bincount(ids, minlength=C).astype(np.float64)
    sums, sumsq, _ = _device_stats(hidden, ids)
    total = _pairwise_loss(counts, sums, sumsq, int(np.asarray(d)))
    return np.array(total, dtype=np.float32)
